# revision 52
# baseline (speedup 1.0000x reference)
"""Trainium2 Bass kernel for nn_ExtendedFILIP (FILIP-style contrastive loss).

Strategy (8 NeuronCores, no collectives):
  - Shard the REC (bB) batch axis: core c handles rec batches [4c, 4c+4).
  - Every core encodes the full PEP set (cheap: 9.7 GFLOP) plus its rec shard,
    computes its block of the pairwise token-similarity matrix twice
    (once [a,s] for the tB-max, once [s,a] for the tA-max; both maxes are then
    free-dim reductions), and returns per-token segment maxes.
  - Host does the final masked means (tiny) and concatenation.

Precision/perf choices (steady-state HW: ~0.54ms/rep vs ~1.0ms for the
all-bf16 predecessor, tracking the CoreSim cost model's 510us vs 789us;
end-to-end relmax vs the fp32 reference ~1.1e-2, gate 2e-2):
  - The projection (47% of encode PE columns) runs fp8e4m3 DoubleRow: x and
    pw are fp8, with pw scaled x64 on the host so its ~0.02-scale values
    clear the fp8 subnormal range, and the exact 1/64 folded into f1w (a
    bf16 exponent shift). f1/f2 stay bf16 (fp8 there measured ~1.7e-2 -
    too close to the gate).
  - The projection is emitted "flipped" (weights stationary, out [emb, tok]),
    so e lands in PSUM already transposed: one PE transpose round and one
    PSUM round-trip per tile eliminated. The oT eviction runs on DVE a full
    step after its transpose (ACT and PE were the encode-phase co-bottleneck
    after the fp8 projection).
  - Normalized embeddings are stored fp8e4m3 and both similarity passes run
    DoubleRow matmuls (K=256 per instruction, 2x PE model rate / ~1.4x HW).
  - Sim-phase max machinery (TensorReduce has no fast DVE mode, so raw fp32
    reduces would make DVE the wall at 1 elem/lane/cycle): the otherwise-idle
    ACT engine evicts the 4 PSUM banks of most groups to fp16, DVE then
    combines chunks with 2-byte 2x TT-max trees before a short merged
    reduce. Pass 0 merges each rec batch's two 512-chunks on-device (sa is
    [128,128] batch maxes); pass 1 folds the 128-wide segments 3x before a
    16-wide reduce, with 1 in 4 groups reducing PSUM directly to balance
    ACT vs DVE (~229us vs ~220us busy in the sim phase).
  - sa/sb outputs are fp16.

Raw Bass (no Tile framework): this toolchain's walrus rejects instructions
with more than one sync-wait, which Tile's scheduler emits freely. All
synchronization below is explicit single-wait semaphore choreography:
each engine carries a monotonically increasing progress semaphore; waits are
standalone single-sem threshold instructions. Cross-engine events are
resolved at emission time (after the whole schedule is built), so waits may
reference events recorded later in build order.
"""

import numpy as np
import ml_dtypes

B, TA, TB, DIN, DEMB = 32, 128, 1024, 1280, 512
NCORES = 8
BSH = B // NCORES            # rec batches per core
NA = B * TA                  # 4096 pep tokens
NB = BSH * TB                # 4096 rec tokens per core
KD = DIN // 128              # 10 K-tiles for the projection
KE = DEMB // 128             # 4 K-tiles for emb-dim contractions
NTILE = NA // 128            # 32 pep token tiles
NTILE_B = NB // 128          # 32 rec token tiles per core
NT = NTILE + NTILE_B         # 64 encode tiles total
NEG = -1.0e30
LN_EPS = 1e-5
MM_EPS = 1e-6
CH = 256                     # tokens per input-activation DMA chunk
NCHUNK = (NA + NB) // CH     # 32 chunks (16 pep then 16 rec)

_BF = ml_dtypes.bfloat16
_F8 = ml_dtypes.float8_e4m3fn


def _build_nc(reps=1):
    import concourse.bass as bass
    import concourse.mybir as mybir

    dt = mybir.dt
    ALU = mybir.AluOpType
    AF = mybir.ActivationFunctionType
    AX = mybir.AxisListType

    nc = bass.Bass()

    # ---------------- DRAM I/O ----------------
    d_xat = nc.dram_tensor("xat", [DIN, NA], dt.float8e4, kind="ExternalInput")
    d_xbt = nc.dram_tensor("xbt", [DIN, NB], dt.float8e4, kind="ExternalInput")
    d_idt = nc.dram_tensor("idt", [128, 128], dt.bfloat16, kind="ExternalInput")
    d_w = {}
    for e in ("a", "b"):
        d_w[e + "pw"] = nc.dram_tensor(e + "pw", [DIN, DEMB], dt.float8e4, kind="ExternalInput")
        d_w[e + "f1w"] = nc.dram_tensor(e + "f1w", [DEMB, DEMB], dt.bfloat16, kind="ExternalInput")
        d_w[e + "f2w"] = nc.dram_tensor(e + "f2w", [DEMB, DEMB], dt.bfloat16, kind="ExternalInput")
        d_w[e + "f1b"] = nc.dram_tensor(e + "f1b", [1, DEMB], dt.bfloat16, kind="ExternalInput")
        d_w[e + "f2b"] = nc.dram_tensor(e + "f2b", [1, DEMB], dt.bfloat16, kind="ExternalInput")
    d_sa = nc.dram_tensor("sa", [128, 128], dt.float16, kind="ExternalOutput")
    d_sb = nc.dram_tensor("sb", [128, 1024], dt.float16, kind="ExternalOutput")

    # ---------------- SBUF ----------------
    s_idt = nc.alloc_sbuf_tensor("s_idt", [128, 128], dt.bfloat16)
    s_pw = {e: nc.alloc_sbuf_tensor(f"s_{e}pw", [128, KD, DEMB], dt.float8e4) for e in "ab"}
    s_f1w = {e: nc.alloc_sbuf_tensor(f"s_{e}f1w", [128, KE, DEMB], dt.bfloat16) for e in "ab"}
    s_f2w = {e: nc.alloc_sbuf_tensor(f"s_{e}f2w", [128, KE, DEMB], dt.bfloat16) for e in "ab"}
    s_f1b = {e: nc.alloc_sbuf_tensor(f"s_{e}f1b", [1, DEMB], dt.bfloat16) for e in "ab"}
    s_f2b = {e: nc.alloc_sbuf_tensor(f"s_{e}f2b", [1, DEMB], dt.bfloat16) for e in "ab"}
    s_ones = nc.alloc_sbuf_tensor("s_ones", [1, 128], dt.bfloat16)
    # Normalized embeddings stored as fp8e4m3: the pairwise-sim matmuls run in
    # DoubleRow perf mode (K=256 per instruction, 2x PE throughput). Host-side
    # emulation puts the fp8 quantization error at relmax ~6e-3 vs the fp32
    # reference (gate is 2e-2); encode stays bf16.
    s_hat = nc.alloc_sbuf_tensor("s_hat", [128, KE, NA], dt.float8e4)
    s_hbt = nc.alloc_sbuf_tensor("s_hbt", [128, KE, NB], dt.float8e4)
    s_xc = [nc.alloc_sbuf_tensor(f"s_xc{i}", [128, KD, CH], dt.float8e4) for i in (0, 1)]
    s_eT = [nc.alloc_sbuf_tensor(f"s_eT{i}", [128, KE, 128], dt.bfloat16) for i in (0, 1)]
    s_h = [nc.alloc_sbuf_tensor(f"s_h{i}", [128, DEMB], dt.float32) for i in (0, 1)]
    s_hn = [nc.alloc_sbuf_tensor(f"s_hn{i}", [128, DEMB], dt.bfloat16) for i in (0, 1)]
    s_hnT = [nc.alloc_sbuf_tensor(f"s_hnT{i}", [128, KE, 128], dt.bfloat16) for i in (0, 1)]
    s_on = [nc.alloc_sbuf_tensor(f"s_on{i}", [128, DEMB], dt.bfloat16) for i in (0, 1)]
    s_scrA = [nc.alloc_sbuf_tensor(f"s_scrA{i}", [128, DEMB], dt.bfloat16) for i in (0, 1)]
    s_scrB = [nc.alloc_sbuf_tensor(f"s_scrB{i}", [128, DEMB], dt.bfloat16) for i in (0, 1)]
    st = {}
    for nm in ("hsum", "hsq", "mu", "varb", "var", "std", "rstd", "osq", "onorm", "rnorm"):
        st[nm] = [nc.alloc_sbuf_tensor(f"s_{nm}{i}", [128, 1], dt.float32) for i in (0, 1)]
    # fp16 outputs (values are cosine sims in [-1,1]; fp16 error ~5e-4 is
    # negligible next to the fp8 sim quantization).
    s_sa = nc.alloc_sbuf_tensor("s_sa", [128, 128], dt.float16)
    # pass-0 staging: ACT evicts the 4 PSUM banks to fp16, DVE combines the
    # two 512-token chunks of each rec batch with a 2-byte 2x TT-max, then one
    # merged reduce produces the batch maxes. Moves half of pass-0's reduce
    # work to the otherwise-idle ACT engine.
    s_sev = [nc.alloc_sbuf_tensor(f"s_sev{i}", [128, 4, 512], dt.float16) for i in range(4)]
    s_scc = [nc.alloc_sbuf_tensor(f"s_scc{i}", [128, 2, 512], dt.float16) for i in range(4)]
    # pass-1 fold tree: TT-max halves the 128-wide segments at the 2-byte 2x
    # rate (64+32 elems/lane) before one 16-wide merged reduce.
    s_sf1 = [nc.alloc_sbuf_tensor(f"s_sf1{i}", [128, 1024], dt.float16) for i in range(4)]
    s_sf2 = [nc.alloc_sbuf_tensor(f"s_sf2{i}", [128, 512], dt.float16) for i in range(4)]
    s_sf3 = [nc.alloc_sbuf_tensor(f"s_sf3{i}", [128, 256], dt.float16) for i in range(4)]
    s_sb = nc.alloc_sbuf_tensor("s_sb", [128, 1024], dt.float16)

    # ---------------- PSUM: 2 tensors of 4 banks each ----------------
    # The sim phase max-reduces a whole 4-bank tensor with ONE DVE
    # tensor_reduce; the encode phase views the same storage as 8 individual
    # [128, 512] banks (slice k of tensor i).
    p_sim = [nc.alloc_psum_tensor(f"ps{i}", [128, 4, 512], dt.float32) for i in (0, 1)]

    def p_slice(i, k):
        return p_sim[i].ap()[:, k, :]

    def p_e(i):
        return p_slice(i, 0)

    def p_h(i):
        return p_slice(i, 1)

    def p_o(i):
        return p_slice(i, 2)

    def pT_bf16(i):
        return p_sim[i].ap().bitcast(dt.bfloat16)[:, 3, :512]

    # ---------------- schedule builder ----------------
    prog = {k: [] for k in ("pe", "act", "dve", "gp")}
    cnt = {"pe": 0, "act": 0, "dve": 0, "din": 0}
    ev = {}                     # event name -> (sem_key, value); resolved at emit time
    cur = {"p": ""}             # event-name prefix (per repetition for benchmarking)

    def emit(engine, fn):
        prog[engine].append(fn)

    def W(engine, event, raw=False):
        event = event if raw else cur["p"] + event

        def f(eng, sems, lw, _e=event):
            if _e not in ev:
                return
            sem_key, val = ev[_e]
            if lw.get(sem_key, 0) >= val:
                return
            lw[sem_key] = val
            eng.wait_ge(sems[sem_key], val)
        emit(engine, f)

    def INC(sem_key, event=None, n=1):
        cnt[sem_key] += n
        if event is not None:
            ev[cur["p"] + event] = (sem_key, cnt[sem_key])
        return cnt[sem_key]

    # ============ gpsimd: all input DMAs (single SWDGE FIFO queue) ============
    def dma_in(dst_fn, src_fn, event=None):
        # Each input DMA is followed by a completion wait on the issuing
        # engine: sem-count prefix waits are only sound when no later DMA on
        # the same semaphore is in flight (out-of-order completion hazard).
        v = INC("din", event, 16)
        emit("gp", lambda eng, sems, lw, _d=dst_fn, _s=src_fn:
             eng.dma_start(out=_d(), in_=_s()).then_inc(sems["din"], 16))
        emit("gp", lambda eng, sems, lw, _v=v: eng.wait_ge(sems["din"], _v))

    # Front-load only what proj[0] needs (idt + pep projection weights);
    # the remaining weight DMAs are emitted after the first two activation
    # chunks so PE doesn't idle ~20us at startup behind the weight queue.
    dma_in(lambda: s_idt.ap()[:, :], lambda: d_idt[:, :])
    dma_in(lambda: s_pw["a"].ap()[:, :, :],
           lambda: d_w["apw"].rearrange("(k p) n -> p k n", p=128))

    def dma_weights_rest():
        # din counts are cumulative, so waiting for the checkpoint events
        # below covers every DMA emitted before them on this queue.
        dma_in(lambda: s_f1w["a"].ap()[:, :, :],
               lambda: d_w["af1w"].rearrange("(k p) n -> p k n", p=128))
        dma_in(lambda: s_f1b["a"].ap()[:, :], lambda: d_w["af1b"][:, :])
        dma_in(lambda: s_f2w["a"].ap()[:, :, :],
               lambda: d_w["af2w"].rearrange("(k p) n -> p k n", p=128))
        dma_in(lambda: s_f2b["a"].ap()[:, :], lambda: d_w["af2b"][:, :],
               event="din_awts")
        dma_in(lambda: s_pw["b"].ap()[:, :, :],
               lambda: d_w["bpw"].rearrange("(k p) n -> p k n", p=128))
        dma_in(lambda: s_f1w["b"].ap()[:, :, :],
               lambda: d_w["bf1w"].rearrange("(k p) n -> p k n", p=128))
        dma_in(lambda: s_f1b["b"].ap()[:, :], lambda: d_w["bf1b"][:, :])
        dma_in(lambda: s_f2w["b"].ap()[:, :, :],
               lambda: d_w["bf2w"].rearrange("(k p) n -> p k n", p=128))
        dma_in(lambda: s_f2b["b"].ap()[:, :], lambda: d_w["bf2b"][:, :],
               event="din_bwts")

    def one_rep():
        nonlocal ngrp
        ngrp = 0
        for c in range(NCHUNK):
            src = d_xat if c < NCHUNK // 2 else d_xbt
            off = (c % (NCHUNK // 2)) * CH
            if c == 2 and cur["p"] == "r0_":
                dma_weights_rest()
            if c >= 2:
                # WAR: buffer c%2 must be fully read by proj of tiles 2(c-2), 2(c-2)+1
                W("gp", f"pe_proj_{2 * (c - 2) + 1}")
            dma_in(lambda c=c: s_xc[c % 2].ap()[:, :, :],
                   lambda src=src, off=off: src.rearrange("(k p) t -> p k t", p=128)[:, :, off:off + CH],
                   event=f"din_chunk_{c}")

        # ============ helpers ============
        def mm(out_fn, lhs_fn, rhs_fn, start, stop, inc_event=None, perf_mode=None):
            def f(eng, sems, lw, _o=out_fn, _l=lhs_fn, _r=rhs_fn, _s=start, _p=stop,
                  _e=inc_event, _pm=perf_mode):
                ins = nc.tensor.matmul(_o(), _l(), _r(), start=_s, stop=_p,
                                       perf_mode=_pm, skip_group_check=True)
                if _e is not None:
                    ins.then_inc(sems["pe"], 1)
            emit("pe", f)
            if inc_event is not None:
                INC("pe", inc_event)

        def act_op(fn, event):
            emit("act", lambda eng, sems, lw, _fn=fn: _fn().then_inc(sems["act"], 1))
            INC("act", event)

        def dve_op(fn, event):
            emit("dve", lambda eng, sems, lw, _fn=fn: _fn().then_inc(sems["dve"], 1))
            INC("dve", event)

        def enc_of(u):
            return "a" if u < NTILE else "b"

        def tok_slice(u):
            return (u % NTILE) * 128

        # ============ encode: 64 token tiles, software pipeline ============
        # PE stage lags within build step s: proj s | f1 s-2 | trH s-3 | f2 s-4 | trO s-5
        # proj is emitted "flipped" (weights stationary, out [emb, tok]), so e
        # lands in PSUM already transposed: no trE round and a single eT evict.
        tr_i = 0                   # global transpose-round counter
        tr_bank = {}               # round -> p_T parity

        def transpose_round(src_fn, inc_event):
            nonlocal tr_i
            r = tr_i
            tr_bank[r] = r % 2
            W("pe", f"ac_evT_{r - 2}")
            for m in range(4):
                def f(eng, sems, lw, _m=m, _src=src_fn, _r=r, _last=(m == 3), _e=inc_event):
                    ins = nc.tensor.transpose(
                        pT_bf16(tr_bank[_r])[:, _m * 128:(_m + 1) * 128],
                        _src()[:, _m * 128:(_m + 1) * 128],
                        s_idt.ap()[:, :],
                    )
                    if _last:
                        ins.then_inc(sems["pe"], 1)
                emit("pe", f)
            INC("pe", inc_event)
            tr_i += 1
            return r

        trH_round, trO_round = {}, {}

        for s in range(NT + 6):
            # ---------------- PE ----------------
            u = s
            if u < NT:  # proj[u], flipped: out[emb_m, tok] = W[:, m]^T @ xT
                W("pe", f"din_chunk_{u // 2}")
                if u == NTILE:
                    W("pe", "din_bwts")   # all rec-side weights landed
                W("pe", f"ac_evict_eT_{u - 2}")
                pb = u % 2
                for m in range(KE):
                    for k2 in range(KD // 2):
                        mm(lambda pb=pb, m=m: p_e(pb)[:, m * 128:(m + 1) * 128],
                           lambda u=u, k2=k2, m=m: s_pw[enc_of(u)].ap()[:, 2 * k2:2 * k2 + 2, m * 128:(m + 1) * 128],
                           lambda u=u, k2=k2: s_xc[(u // 2) % 2].ap()[:, 2 * k2:2 * k2 + 2, (u % 2) * 128:(u % 2) * 128 + 128],
                           start=(k2 == 0), stop=(k2 == KD // 2 - 1),
                           inc_event=(f"pe_proj_{u}" if (k2 == KD // 2 - 1 and m == KE - 1) else None),
                           perf_mode=mybir.MatmulPerfMode.DoubleRow)
            u = s - 2
            if 0 <= u < NT:  # f1[u]
                W("pe", f"ac_evict_eT_{u}")
                W("pe", f"ac_relu_{u - 2}")
                if u == 0:
                    W("pe", "dv_ones")
                    W("pe", "din_awts")   # a-side f1/f2 weights landed
                pb = u % 2
                for k in range(KE):
                    mm(lambda pb=pb: p_h(pb),
                       lambda u=u, k=k: s_eT[u % 2].ap()[:, k, :],
                       lambda u=u, k=k: s_f1w[enc_of(u)].ap()[:, k, :],
                       start=(k == 0), stop=False)
                mm(lambda pb=pb: p_h(pb),
                   lambda: s_ones.ap()[:, :],
                   lambda u=u: s_f1b[enc_of(u)].ap()[:, :],
                   start=False, stop=True, inc_event=f"pe_f1_{u}")
            u = s - 3
            if 0 <= u < NT:  # trH[u]
                W("pe", f"dv_lnapply_{u}")
                trH_round[u] = transpose_round(lambda u=u: s_hn[u % 2].ap(), f"pe_trH_{u}")
            u = s - 4
            if 0 <= u < NT:  # f2[u]
                W("pe", f"dv_evict_hnT_{u}")
                W("pe", f"dv_normapply_{u - 2}")
                W("pe", f"ac_l2ss_{u - 2}")
                pb = u % 2
                for k in range(KE):
                    mm(lambda pb=pb: p_o(pb),
                       lambda u=u, k=k: s_hnT[u % 2].ap()[:, k, :],
                       lambda u=u, k=k: s_f2w[enc_of(u)].ap()[:, k, :],
                       start=(k == 0), stop=False)
                mm(lambda pb=pb: p_o(pb),
                   lambda: s_ones.ap()[:, :],
                   lambda u=u: s_f2b[enc_of(u)].ap()[:, :],
                   start=False, stop=True, inc_event=f"pe_f2_{u}")
            u = s - 5
            if 0 <= u < NT:  # trO[u]
                W("pe", f"dv_normapply_{u}")
                trO_round[u] = transpose_round(lambda u=u: s_on[u % 2].ap(), f"pe_trO_{u}")

            # ---------------- ACT ----------------
            u = s
            if u < NT:  # evict eT: psum fp32 [emb, tok] -> sbuf bf16 (single copy)
                W("act", f"pe_proj_{u}")
                W("act", f"pe_f1_{u - 2}")
                act_op(lambda u=u: nc.scalar.copy(
                    s_eT[u % 2].ap()[:, :, :],
                    p_e(u % 2).rearrange("p (c x) -> p c x", x=128)),
                    f"ac_evict_eT_{u}")
            u = s - 2
            if 0 <= u < NT:  # relu + per-token sum
                W("act", f"pe_f1_{u}")
                W("act", f"dv_lnapply_{u - 2}")
                act_op(lambda u=u: nc.scalar.activation(
                    s_h[u % 2].ap()[:, :], p_h(u % 2),
                    AF.Relu, accum_out=st["hsum"][u % 2].ap()[:, :]),
                    f"ac_relu_{u}")
                # sum of squares of relu'd h (same-engine RAW: self-wait)
                W("act", f"ac_relu_{u}")
                W("act", f"ac_hsq_{u - 2}")
                act_op(lambda u=u: nc.scalar.activation(
                    s_scrA[u % 2].ap()[:, :], s_h[u % 2].ap()[:, :],
                    AF.Square, accum_out=st["hsq"][u % 2].ap()[:, :]),
                    f"ac_hsq_{u}")
            u = s - 2
            if 0 <= u < NT:  # std = sqrt(var)  (eps already folded into var)
                W("act", f"dv_var_{u}")
                act_op(lambda u=u: nc.scalar.activation(
                    st["std"][u % 2].ap()[:, :], st["var"][u % 2].ap()[:, :],
                    AF.Sqrt, bias=0.0),
                    f"ac_std_{u}")
            u = s - 5
            u = s - 4
            if 0 <= u < NT:  # l2 sum of squares from psum_o (Square + accum)
                W("act", f"pe_f2_{u}")
                W("act", f"ac_l2ss_{u - 2}")
                act_op(lambda u=u: nc.scalar.activation(
                    s_scrB[u % 2].ap()[:, :], p_o(u % 2),
                    AF.Square, accum_out=st["osq"][u % 2].ap()[:, :]),
                    f"ac_l2ss_{u}")
                W("act", f"ac_l2ss_{u}")
                act_op(lambda u=u: nc.scalar.activation(
                    st["onorm"][u % 2].ap()[:, :], st["osq"][u % 2].ap()[:, :],
                    AF.Sqrt, bias=0.0),
                    f"ac_onorm_{u}")

            # ---------------- DVE ----------------
            if s == 0:
                dve_op(lambda: nc.vector.memset(s_ones.ap()[:, :], 1.0), "dv_ones")
            u = s - 3
            if 0 <= u < NT:  # lnapply: hn = (h - mu) * rstd
                W("dve", f"ac_std_{u}")
                W("dve", f"pe_trH_{u - 2}")
                W("dve", f"dv_mu_{u}")
                dve_op(lambda u=u: nc.vector.reciprocal(
                    st["rstd"][u % 2].ap()[:, :], st["std"][u % 2].ap()[:, :]),
                    f"dv_rstd_{u}")
                W("dve", f"dv_rstd_{u}")
                dve_op(lambda u=u: nc.vector.tensor_scalar(
                    s_hn[u % 2].ap()[:, :], s_h[u % 2].ap()[:, :],
                    st["mu"][u % 2].ap()[:, :], st["rstd"][u % 2].ap()[:, :],
                    ALU.subtract, ALU.mult),
                    f"dv_lnapply_{u}")
            u = s - 4
            if 0 <= u < NT:  # evict hnT: psum (bf16 bitcast, 2-byte 2x copy) -> sbuf
                W("dve", f"pe_trH_{u}")
                W("dve", f"pe_f2_{u - 2}")
                dve_op(lambda u=u: nc.vector.tensor_copy(
                    s_hnT[u % 2].ap()[:, :, :],
                    pT_bf16(tr_bank[trH_round[u]]).rearrange("p (c x) -> p c x", x=128)),
                    f"dv_evict_hnT_{u}")
                ev[f"{cur['p']}ac_evT_{trH_round[u]}"] = ev[f"{cur['p']}dv_evict_hnT_{u}"]
            u = s - 6
            if 0 <= u < NT:  # evict oT into hat/hbt (DVE; trO landed a full
                # step earlier so this never head-blocks the DVE stream)
                W("dve", f"pe_trO_{u}")
                dst = s_hat if u < NTILE else s_hbt
                dve_op(lambda u=u, dst=dst: nc.vector.tensor_copy(
                    dst.ap()[:, :, tok_slice(u):tok_slice(u) + 128],
                    pT_bf16(tr_bank[trO_round[u]]).rearrange("p (c x) -> p c x", x=128)),
                    f"dv_evict_oT_{u}")
                ev[f"{cur['p']}ac_evT_{trO_round[u]}"] = ev[f"{cur['p']}dv_evict_oT_{u}"]
            u = s - 5
            if 0 <= u < NT:  # normapply: on = psum_o * rnorm
                W("dve", f"ac_onorm_{u}")
                W("dve", f"pe_trO_{u - 2}")
                dve_op(lambda u=u: nc.vector.reciprocal(
                    st["rnorm"][u % 2].ap()[:, :], st["onorm"][u % 2].ap()[:, :]),
                    f"dv_rnorm_{u}")
                W("dve", f"dv_rnorm_{u}")
                dve_op(lambda u=u: nc.vector.tensor_scalar(
                    s_on[u % 2].ap()[:, :], p_o(u % 2),
                    st["rnorm"][u % 2].ap()[:, :], None,
                    ALU.mult),
                    f"dv_normapply_{u}")
            u = s - 2
            if 0 <= u < NT:  # stats: mu, var (hsum/hsq accumulated by ACT)
                W("dve", f"ac_hsq_{u}")
                dve_op(lambda u=u: nc.vector.tensor_scalar(
                    st["mu"][u % 2].ap()[:, :], st["hsum"][u % 2].ap()[:, :],
                    1.0 / DEMB, None, ALU.mult),
                    f"dv_mu_{u}")
                W("dve", f"dv_mu_{u}")
                dve_op(lambda u=u: nc.vector.tensor_scalar(
                    st["varb"][u % 2].ap()[:, :], st["mu"][u % 2].ap()[:, :],
                    st["mu"][u % 2].ap()[:, :], LN_EPS, ALU.mult, ALU.subtract),
                    f"dv_varb_{u}")
                W("dve", f"dv_varb_{u}")
                dve_op(lambda u=u: nc.vector.tensor_scalar(
                    st["var"][u % 2].ap()[:, :], st["hsq"][u % 2].ap()[:, :],
                    1.0 / DEMB, st["varb"][u % 2].ap()[:, :],
                    ALU.mult, ALU.subtract),
                    f"dv_var_{u}")

        # ============ sim passes ============
        ngrp = 0

        evict_path = {}

        def sim_group(pass_i, i, g):
            nonlocal ngrp
            n = ngrp
            # pass 0 and 3 of 4 pass-1 groups go through the fp16 evict+fold
            # path (banks freed by the ACT evicts); the rest reduce PSUM
            # directly on DVE (banks freed by the merged reduce). The mix
            # balances ACT vs DVE work in the sim phase.
            # 3/4 of pass-1 groups through ACT-evict+fold, rest direct on DVE:
            # balances sim-phase ACT vs DVE (measured best mix in the model)
            evict_path[n] = (pass_i == 0) or (n % 4 != 3)
            if n >= 2:
                if evict_path[n - 2]:
                    W("pe", f"ac_sev_{n - 2}_1")
                else:
                    W("pe", f"dv_simred_{n - 2}")
            if n < 2:
                # first use of each parity: all encode PSUM traffic must be done
                W("pe", f"dv_evict_oT_{NT - 1}")
            lhs_src = s_hat if pass_i == 0 else s_hbt
            rhs_src = s_hbt if pass_i == 0 else s_hat
            # fp8 DoubleRow: each matmul consumes 2 K-blocks (K=256), so the
            # 512-deep contraction is 2 instructions per bank instead of 4.
            for k2 in range(KE // 2):
                for cc in range(4):
                    chunk = g * 4 + cc
                    mm(lambda n=n, cc=cc: p_sim[n % 2].ap()[:, cc, :],
                       lambda k2=k2, i=i, lhs_src=lhs_src: lhs_src.ap()[:, 2 * k2:2 * k2 + 2, i * 128:(i + 1) * 128],
                       lambda k2=k2, chunk=chunk, rhs_src=rhs_src: rhs_src.ap()[:, 2 * k2:2 * k2 + 2, chunk * 512:(chunk + 1) * 512],
                       start=(k2 == 0), stop=(k2 == KE // 2 - 1),
                       inc_event=(f"pe_sim_{n}" if (k2 == KE // 2 - 1 and cc == 3) else None),
                       perf_mode=mybir.MatmulPerfMode.DoubleRow)
            # ONE merged max-reduce over all 4 banks of the group: [128,4,512]
            # (pass 0: out 4 cols of sa) or [128,4,4,128] (pass 1: out 16 cols
            # of sb). Saves 3 instruction overheads per group vs per-bank
            # reduces; TensorReduce has no 2x mode so PSUM-direct is optimal.
            if pass_i == 0:
                # ACT: two wide fp16 evicts (one per rec batch = bank pair);
                # sev parity WAR vs group n-2 is implied by its dv_simred.
                W("act", f"pe_sim_{n}")
                W("act", f"dv_simred_{n - 4}")
                for b in (0, 1):
                    act_op(lambda n=n, b=b: nc.scalar.copy(
                        s_sev[n % 4].ap()[:, 2 * b:2 * b + 2, :],
                        p_sim[n % 2].ap()[:, 2 * b:2 * b + 2, :]),
                        f"ac_sev_{n}_{b}")
                # DVE: per batch, TT-max the two chunks at the 2-byte 2x rate,
                # then one merged reduce over [128, 2, 512] -> 2 sa cols.
                for b in (0, 1):
                    W("dve", f"ac_sev_{n}_{b}")
                    dve_op(lambda n=n, b=b: nc.vector.tensor_tensor(
                        s_scc[n % 4].ap()[:, b, :],
                        s_sev[n % 4].ap()[:, 2 * b, :],
                        s_sev[n % 4].ap()[:, 2 * b + 1, :],
                        ALU.max),
                        f"dv_scc_{n}_{b}")
                dve_op(lambda n=n, i=i, g=g: nc.vector.tensor_reduce(
                    s_sa.ap()[:, i * 4 + g * 2:i * 4 + g * 2 + 2],
                    s_scc[n % 4].ap()[:, :, :],
                    AX.X, ALU.max),
                    f"dv_simred_{n}")
            elif evict_path[n]:
                W("act", f"pe_sim_{n}")
                W("act", f"dv_simred_{n - 4}")
                for b in (0, 1):
                    act_op(lambda n=n, b=b: nc.scalar.copy(
                        s_sev[n % 4].ap()[:, 2 * b:2 * b + 2, :],
                        p_sim[n % 2].ap()[:, 2 * b:2 * b + 2, :]),
                        f"ac_sev_{n}_{b}")
                for b in (0, 1):
                    W("dve", f"ac_sev_{n}_{b}")
                    dve_op(lambda n=n, b=b: nc.vector.tensor_tensor(
                        s_sf1[n % 4].ap().rearrange("p (b q x) -> p b q x", b=4, q=4, x=64)[:, 2 * b:2 * b + 2, :, :],
                        s_sev[n % 4].ap().rearrange("p b (q x) -> p b q x", x=128)[:, 2 * b:2 * b + 2, :, 0:64],
                        s_sev[n % 4].ap().rearrange("p b (q x) -> p b q x", x=128)[:, 2 * b:2 * b + 2, :, 64:128],
                        ALU.max),
                        f"dv_f1_{n}_{b}")
                dve_op(lambda n=n: nc.vector.tensor_tensor(
                    s_sf2[n % 4].ap().rearrange("p (b q x) -> p b q x", b=4, q=4, x=32),
                    s_sf1[n % 4].ap().rearrange("p (b q x) -> p b q x", b=4, q=4, x=64)[:, :, :, 0:32],
                    s_sf1[n % 4].ap().rearrange("p (b q x) -> p b q x", b=4, q=4, x=64)[:, :, :, 32:64],
                    ALU.max),
                    f"dv_f2_{n}")
                dve_op(lambda n=n: nc.vector.tensor_tensor(
                    s_sf3[n % 4].ap().rearrange("p (b q x) -> p b q x", b=4, q=4, x=16),
                    s_sf2[n % 4].ap().rearrange("p (b q x) -> p b q x", b=4, q=4, x=32)[:, :, :, 0:16],
                    s_sf2[n % 4].ap().rearrange("p (b q x) -> p b q x", b=4, q=4, x=32)[:, :, :, 16:32],
                    ALU.max),
                    f"dv_f3_{n}")
                dve_op(lambda n=n, i=i, g=g: nc.vector.tensor_reduce(
                    s_sb.ap()[:, i * 32 + g * 16:i * 32 + g * 16 + 16],
                    s_sf3[n % 4].ap().rearrange("p (b q x) -> p b q x", b=4, q=4, x=16),
                    AX.X, ALU.max),
                    f"dv_simred_{n}")
            else:
                W("dve", f"pe_sim_{n}")
                dve_op(lambda n=n, i=i, g=g: nc.vector.tensor_reduce(
                    s_sb.ap()[:, i * 32 + g * 16:i * 32 + g * 16 + 16],
                    p_sim[n % 2].ap()[:, :, :].rearrange("p b (q x) -> p b q x", x=128),
                    AX.X, ALU.max),
                    f"dv_simred_{n}")
            ngrp += 1

        for i in range(NTILE):
            for g in range(2):
                sim_group(0, i, g)
        for j in range(NTILE_B):
            for g in range(2):
                sim_group(1, j, g)

    ngrp = 0
    for rep in range(reps):
        cur["p"] = f"r{rep}_"
        if rep:
            for engk in ("gp", "pe", "act", "dve"):
                W(engk, f"r{rep - 1}_END", raw=True)
        one_rep()
        ev[f"r{rep}_END"] = ("dve", cnt["dve"])

    last_dv = cnt["dve"]

    # ---------------- emit per-engine programs ----------------
    with (
        nc.semaphore("sem_din") as sem_din,
        nc.semaphore("sem_dout") as sem_dout,
        nc.semaphore("sem_pe") as sem_pe,
        nc.semaphore("sem_act") as sem_act,
        nc.semaphore("sem_dve") as sem_dve,
        nc.Block() as block,
    ):
        sems = {"din": sem_din, "dout": sem_dout, "pe": sem_pe, "act": sem_act, "dve": sem_dve}

        @block.gpsimd
        def _(g):
            lw = {}
            for f in prog["gp"]:
                f(g, sems, lw)

        @block.tensor
        def _(t):
            lw = {}
            for f in prog["pe"]:
                f(t, sems, lw)

        @block.scalar
        def _(a):
            lw = {}
            for f in prog["act"]:
                f(a, sems, lw)

        @block.vector
        def _(v):
            lw = {}
            for f in prog["dve"]:
                f(v, sems, lw)

        @block.sync
        def _(sy):
            sy.wait_ge(sems["dve"], last_dv)
            sy.dma_start(out=d_sa[:, :], in_=s_sa.ap()[:, :]).then_inc(sems["dout"], 16)
            sy.dma_start(out=d_sb[:, :], in_=s_sb.ap()[:, :]).then_inc(sems["dout"], 16)
            sy.wait_ge(sems["dout"], 32)

    return nc


# ---------------- host side ----------------

def _fold_params(inputs, pre):
    f32 = np.float32
    pw = np.asarray(inputs[pre + "_pw"], f32)
    pb = np.asarray(inputs[pre + "_pb"], f32)
    f1w = np.asarray(inputs[pre + "_f1w"], f32)
    f1b = np.asarray(inputs[pre + "_f1b"], f32)
    lng = np.asarray(inputs[pre + "_lng"], f32)
    lnb = np.asarray(inputs[pre + "_lnb"], f32)
    f2w = np.asarray(inputs[pre + "_f2w"], f32)
    f2b = np.asarray(inputs[pre + "_f2b"], f32)
    f1b_eff = f1b + pb @ f1w
    f2w_eff = lng[:, None] * f2w
    f2b_eff = f2b + lnb @ f2w
    # proj runs in fp8e4m3: pw values (~0.02 scale) would mostly land in fp8
    # subnormals, so scale them x64 into the normal range and fold the exact
    # 1/64 (a bf16 exponent shift) into f1w.
    return (np.ascontiguousarray(pw * 64.0).astype(_F8),
            np.ascontiguousarray(f1w / 64.0).astype(_BF),
            np.ascontiguousarray(f2w_eff).astype(_BF),
            np.ascontiguousarray(f1b_eff[None, :]).astype(_BF),
            np.ascontiguousarray(f2b_eff[None, :]).astype(_BF))


def _dup_valid_tokens(x, mask):
    """Replace masked-out tokens' feature vectors with a valid token's vector
    from the same batch. Max over the batch's tokens is unchanged by
    duplicates, so the device needs no mask bias at all. Batches with no valid
    token are left untouched (probability ~2^-T with random masks)."""
    x = np.array(x, np.float32, copy=True)   # [b, t, d]
    m = mask.astype(bool)
    for b in range(x.shape[0]):
        valid = np.flatnonzero(m[b])
        if valid.size and valid.size < x.shape[1]:
            x[b, ~m[b]] = x[b, valid[0]]
    return x


def _masked_mean(t, mask):
    num = np.where(mask, t, 0.0).sum(-1, dtype=np.float32)
    den = np.maximum(mask.sum(-1).astype(np.float32), MM_EPS)
    return (num / den).astype(np.float32)


_NC_CACHE = {}


def kernel(pep_esm, rec_esm, pep_mask, rec_mask, temperature,
           pep_pw, pep_pb, pep_f1w, pep_f1b, pep_lng, pep_lnb, pep_f2w, pep_f2b,
           rec_pw, rec_pb, rec_f1w, rec_f1b, rec_lng, rec_lnb, rec_f2w, rec_f2b):
    from concourse.bass_utils import run_bass_kernel_spmd

    inputs = dict(pep_pw=pep_pw, pep_pb=pep_pb, pep_f1w=pep_f1w, pep_f1b=pep_f1b,
                  pep_lng=pep_lng, pep_lnb=pep_lnb, pep_f2w=pep_f2w, pep_f2b=pep_f2b,
                  rec_pw=rec_pw, rec_pb=rec_pb, rec_f1w=rec_f1w, rec_f1b=rec_f1b,
                  rec_lng=rec_lng, rec_lnb=rec_lnb, rec_f2w=rec_f2w, rec_f2b=rec_f2b)

    if "nc" not in _NC_CACHE:
        _NC_CACHE["nc"] = _build_nc()
    nc = _NC_CACHE["nc"]

    f32 = np.float32
    apw, af1w, af2w, af1b, af2b = _fold_params(inputs, "pep")
    bpw, bf1w, bf2w, bf1b, bf2b = _fold_params(inputs, "rec")

    mA = np.asarray(pep_mask).astype(bool)
    mB = np.asarray(rec_mask).astype(bool)
    pep_eff = _dup_valid_tokens(np.asarray(pep_esm, f32), mA)
    rec_eff = _dup_valid_tokens(np.asarray(rec_esm, f32), mB)

    xat = np.ascontiguousarray(pep_eff.reshape(NA, DIN).T).astype(_F8)
    idt = np.eye(128, dtype=_BF)

    in_maps = []
    for c in range(NCORES):
        shard = rec_eff[c * BSH:(c + 1) * BSH].reshape(NB, DIN)
        xbt = np.ascontiguousarray(shard.T).astype(_F8)
        in_maps.append({
            "xat": xat, "xbt": xbt, "idt": idt,
            "apw": apw, "af1w": af1w, "af2w": af2w, "af1b": af1b, "af2b": af2b,
            "bpw": bpw, "bf1w": bf1w, "bf2w": bf2w, "bf1b": bf1b, "bf2b": bf2b,
        })

    _NC_CACHE["last_in_maps"] = in_maps
    res = run_bass_kernel_spmd(nc, in_maps, core_ids=list(range(NCORES)))

    temp = float(np.asarray(temperature))
    sA = np.empty((B, B, TA), f32)   # [bA, bB, tA]
    sB = np.empty((B, B, TB), f32)   # [bA, bB, tB]
    for c in range(NCORES):
        ra = np.asarray(res.results[c]["sa"], f32)       # [128, 128]
        rb = np.asarray(res.results[c]["sb"], f32)       # [128, 1024]
        va = ra.reshape(128, NTILE, 4)                   # [tok, bA, bB_local]
        sA[:, c * BSH:(c + 1) * BSH, :] = va.transpose(1, 2, 0)
        vb = rb.reshape(128, NTILE_B, 32)                # [tok, s_tile, bA]
        for j in range(NTILE_B):
            bB = c * BSH + j // 8
            ts0 = (j % 8) * 128
            sB[:, bB, ts0:ts0 + 128] = vb[:, j].T
    scores_A = (_masked_mean(sA, mA[:, None, :]) / temp).astype(f32)
    scores_B = (_masked_mean(sB, mB[None, :, :]) / temp).astype(f32)
    return scores_A, scores_B

